# revision 47
# baseline (speedup 1.0000x reference)
"""Bilateral filter (5x5, sigmaXY=sigmaZ=1) on 8 Trainium2 NeuronCores.

Math (p neighbor, c center, both in [0,1)):
    sim(p,c) = w_spatial * exp(-0.5(p-c)^2)
             = w_spatial * t0(p) * e^{z} * (c-side factors that cancel in the ratio)
    with z = (p-1/2)(c-1/2) in [-1/4,1/4],  t0 = e^{-(p-1/2)^2/2 + 1/8}
Degree-1 weighted-LS fit  e^z ~= a0 + a1 z  gives (t_k = t0*(p-1/2)^k):
    den = S0 + c'*S1,   M = S1 + c'*S2,   S_k = gauss5x5 (*) t_k
    out = 1/2 + M/den            (a1/a0 = 1.00018 absorbed, error ~1e-4)

Engine split per 104-col chunk (x on partitions, (img,y) free):
  ACT   sq=Square(x-.5), t0=Exp, evac S0,S1 (PSUM->fp16 SBUF)
  Pool  t1=(x-.5)*t0 -> fp8, t2=(x-.5)*t1 -> fp8, qm=(x_c-.5)*S2psum
  PE    S0: 10 fp16 matmuls; S1,S2: 10 fp8 DoubleRow matmuls each at
        0.5 cyc/row (second k-tile = step-0 moving rows x e4m3 residual
        of the band weights -> ~fp11 weight precision for free)
  DVE   u1=x_c-.5, qd, den, Newton reciprocal (linear seed, 1 step),
        em, out' = em*(-rden)
Output fp16 y' = -(out-1/2); host computes 1/2 - y'.

Measured l2 rel err vs fp32 reference: ~4e-3 (gate 2e-2).
"""

import numpy as np
from contextlib import ExitStack

import concourse.bass as bass
import concourse.bacc as bacc
import concourse.tile as tile
from concourse import mybir
from concourse.bass import AP
from concourse.bass_utils import run_bass_kernel_spmd
import ml_dtypes

F32 = mybir.dt.float32
F16 = mybir.dt.float16
F8 = mybir.dt.float8e4
NP_F16 = np.float16
NP_F8 = ml_dtypes.float8_e4m3

N_CORES = 8
NIMG = 12            # 4 batch * 3 channels
H = 512
W = 512
ROWS = 64            # output rows per core
R = ROWS + 4         # input rows per core incl halo
WPAD = 524           # 512 + 2+2 conv pad + 8 slack for 5*104 chunking
NCHUNK = 5
CH_OUT = 104         # output cols per chunk
CH_IN = CH_OUT + 4   # input cols per chunk
M8 = 112             # fp8 stationary col count (16-aligned), 104 useful
GRP = 6              # imgs per matmul group (contiguous flat moving)
NMOV = GRP * R - 4   # 404: moving rows per fp8 matmul (incl 4*? junk cols)

DEGREE = 1           # kept for test.py compat (cache key)

# Newton seed for 1/den on den in [4.4, 8.8] (hard bounds of S0 + c'S1)
_RA, _RB = 4.4, 8.8
_NB = 2.0 / (_RA * _RB + (_RA + _RB) ** 2 / 4.0)
_NA = (_RA + _RB) * _NB

# engine-assignment flags
QM_ON_POOL = False   # qm on Pool needs an SBUF S2 (GPSIMD cannot read PSUM)
QM_EVAC = True       # evacuate S2 on ACT, qm as fp16 TT on DVE (balance DVE/ACT)
PREFETCH = 2         # chunks of input DMAs issued ahead of the store queue
SQ_ON_ACT = True     # sq via ACT Square (else DVE pm/mul)
NEWTON2 = False      # second Newton step for 1/den
RECIP_LNEXP = ()  # chunks whose 1/den runs as ACT ln+exp (ACT idles late)

_W1D = np.exp(-0.5 * np.array([4.0, 1.0, 0.0, 1.0, 4.0], dtype=np.float64)).astype(
    np.float32
)


def _e4m3(a):
    return np.asarray(a, np.float32).astype(NP_F8).astype(np.float32)


def _build_bands16() -> np.ndarray:
    """b16[q, dy, o] = wx[q-o] * wy[dy] for q-o in [0,4], else 0 (fp16)."""
    b = np.zeros((CH_IN, 5, CH_OUT), dtype=np.float32)
    for o in range(CH_OUT):
        for d in range(5):
            b[o + d, :, o] = _W1D[d] * _W1D
    return b.astype(NP_F16)


def _build_bands8() -> np.ndarray:
    """b8[q, dy, kt, o]: e4m3 band + e4m3 residual in the second k-tile."""
    b = np.zeros((CH_IN, 5, 2, M8), dtype=np.float32)
    for o in range(CH_OUT):
        for d in range(5):
            for dy in range(5):
                w = np.float32(_W1D[d] * _W1D[dy])
                w0 = _e4m3(w)
                b[o + d, dy, 0, o] = w0
                b[o + d, dy, 1, o] = _e4m3(w - w0)
    return b.astype(NP_F8)


def build_nc(degree: int = DEGREE, bench_iters: int = 1):
    nc = bacc.Bacc("TRN2", target_bir_lowering=False)
    const_tensors = []
    for v in (-0.5, 0.125, 2.0, 1.0, 0.0, -1.0):
        t_ = nc.alloc_sbuf_tensor(f"const-f32-{v}", [128, 1], F32)
        nc.const_aps.aps[(F32, v)] = t_.ap()
        const_tensors.append((t_, v))
    x_d = nc.dram_tensor("x", [WPAD, NIMG, R], F16, kind="ExternalInput")
    b16_d = nc.dram_tensor("b16", [CH_IN, 5, CH_OUT], F16, kind="ExternalInput")
    b8_d = nc.dram_tensor("b8", [CH_IN, 5, 2, M8], F8, kind="ExternalInput")
    y_d = nc.dram_tensor("y", [WPAD, NIMG, ROWS], F16, kind="ExternalOutput")

    AOP = mybir.AluOpType

    with ExitStack() as ctx:
        tc = ctx.enter_context(tile.TileContext(nc))
        singles = ctx.enter_context(tc.tile_pool(name="singles", bufs=1))
        fields = ctx.enter_context(tc.tile_pool(name="fields", bufs=3))
        evac = ctx.enter_context(tc.tile_pool(name="evac", bufs=3))
        asm = ctx.enter_context(tc.tile_pool(name="asm", bufs=3))
        psum = ctx.enter_context(tc.tile_pool(name="psum", bufs=1, space="PSUM"))

        for t_, v in const_tensors:
            nc.gpsimd.memset(t_.ap(), v)
        b16 = singles.tile([CH_IN, 5, CH_OUT], F16)
        b8 = singles.tile([CH_IN, 5, 2, M8], F8)

        def mov8(t, g, dy):
            """[108, 2(step 0), 404] moving AP into field tile t at group g, dy."""
            full = t[:]
            ap0 = [list(d) for d in full.ap][0]
            off = full.offset + g * (GRP * R) + dy
            return AP(full.tensor, off, [ap0, [0, 2], [1, NMOV]])

        def psum_view(pt):
            """[104, 2, 6, 64] useful-col view of fp8-conv psum [112, 2, 512]."""
            full = pt[:]
            ap0 = [list(d) for d in full.ap][0]
            ap0 = [ap0[0], CH_OUT]
            return AP(full.tensor, full.offset, [ap0, [512, 2], [R, GRP], [1, ROWS]])

        def body():
            xts, xcs, flds = {}, {}, {}

            def load_x(j):
                c0 = CH_OUT * j
                x_t = fields.tile([CH_IN, NIMG, R], F16, name="x_t", tag="x_t",
                                  bufs=NCHUNK)
                nc.sync.dma_start(out=x_t, in_=x_d[c0 : c0 + CH_IN])
                xts[j] = x_t

            def load_xc(j):
                c0 = CH_OUT * j
                x_cf = fields.tile([CH_OUT, 2, GRP, R], F16, name="x_c",
                                   tag="x_c", bufs=NCHUNK)
                nc.sync.dma_start(out=x_cf, in_=x_d[c0 + 2 : c0 + 2 + CH_OUT])
                xcs[j] = x_cf[:, :, :, 2 : 2 + ROWS]

            def load_fields(j):
                x_t = xts[j]
                sq = fields.tile([CH_IN, NIMG, R], F16, name="sq", tag="sq",
                                 bufs=NCHUNK)
                pm = fields.tile([CH_IN, NIMG, R], F16, name="pm", tag="pm",
                                 bufs=NCHUNK)
                nc.vector.tensor_scalar_add(pm, x_t, -0.5)
                if j >= 3:
                    nc.scalar.activation(
                        out=sq, in_=x_t, func=mybir.ActivationFunctionType.Square,
                        bias=-0.5, scale=1.0,
                    )
                else:
                    nc.vector.tensor_mul(sq, pm, pm)
                t0 = fields.tile([CH_IN, NIMG, R], F16, name="t0", tag="t0",
                                 bufs=NCHUNK)
                nc.scalar.activation(
                    out=t0, in_=sq, func=mybir.ActivationFunctionType.Exp,
                    bias=0.125, scale=-0.5,
                )
                t1 = fields.tile([CH_IN, NIMG, R], F8, name="t1", tag="t1",
                                 bufs=NCHUNK)
                t2 = fields.tile([CH_IN, NIMG, R], F8, name="t2", tag="t2",
                                 bufs=NCHUNK)
                eng = nc.vector if j <= 1 else nc.gpsimd
                eng.tensor_mul(t1, t0, pm)
                eng.tensor_mul(t2, t0, sq)
                flds[j] = (t0, t1, t2)

            def conv_stage(j):
                t0, t1, t2 = flds[j]
                ps0 = psum.tile([CH_OUT, 2, 8, ROWS], F32, name="ps0", tag="ps0",
                                bufs=2)
                for g in range(2):
                    for dy in range(5):
                        nc.tensor.matmul(
                            ps0[:, g, 0:GRP, :],
                            b16[:, dy, :],
                            t0[:, GRP * g : GRP * (g + 1), dy : dy + ROWS],
                            start=(dy == 0),
                            stop=(dy == 4),
                        )
                ps12 = []
                for k, tk in ((1, t1), (2, t2)):
                    pt = psum.tile([M8, 2, 512], F32, name=f"ps{k}", tag=f"ps{k}")
                    for g in range(2):
                        for dy in range(5):
                            nc.tensor.matmul(
                                pt[:, g, 0:NMOV],
                                b8[:, dy, :, :],
                                mov8(tk, g, dy),
                                start=(dy == 0),
                                stop=(dy == 4),
                                perf_mode=mybir.MatmulPerfMode.DoubleRow,
                            )
                    ps12.append(pt)
                return ps0, ps12

            sh = [CH_OUT, 2, GRP, ROWS]

            def evac_stage(j, ps0, ps12):
                s0e = evac.tile(sh, F16, name="s0e", tag="s0e")
                nc.scalar.copy(out=s0e, in_=ps0[:, :, 0:GRP, :])
                s1e = evac.tile(sh, F16, name="s1e", tag="s1e")
                nc.scalar.copy(out=s1e, in_=psum_view(ps12[0]))
                if QM_EVAC and j >= 2:
                    s2e = evac.tile(sh, F16, name="s2e", tag="s2e")
                    nc.scalar.copy(out=s2e, in_=psum_view(ps12[1]))
                else:
                    s2e = None
                return s0e, s1e, s2e

            def asm_stage(j, s0e, s1e, s2e, ps0, ps12, halves=False):
                c0 = CH_OUT * j
                n_out = min(CH_OUT, W - c0)
                x_c = xcs[j]
                if halves:
                    for g in range(2):
                        _asm_half(j, s0e, s1e, s2e, ps0, ps12, np.s_[:, g : g + 1], g)
                    return
                _asm_half(j, s0e, s1e, s2e, ps0, ps12, np.s_[:, :], None)

            def _asm_half(j, s0e, s1e, s2e, ps0, ps12, sl, g):
                c0 = CH_OUT * j
                n_out = min(CH_OUT, W - c0)
                x_c = xcs[j]
                u1 = asm.tile(sh, F16, name="u1", tag="u1")
                nc.vector.tensor_scalar_add(u1[sl], x_c[sl], -0.5)
                qde = nc.vector
                qd = asm.tile(sh, F16, name="qd", tag="qd")
                den = asm.tile(sh, F16, name="den", tag="den")
                if s0e is None:
                    nc.vector.tensor_mul(qd[sl], u1[sl], psum_view(ps12[0]))
                    nc.vector.scalar_tensor_tensor(
                        den[sl], ps0[:, :, 0:GRP, :], 1.0, qd[sl],
                        AOP.mult, AOP.add,
                    )
                else:
                    qde.tensor_mul(qd[sl], u1[sl], s1e[sl])
                    nc.vector.tensor_add(den[sl], s0e[sl], qd[sl])
                y0 = asm.tile(sh, F16, name="y0", tag="y0")
                nc.vector.tensor_scalar(y0[sl], den[sl], -_NB, _NA, AOP.mult, AOP.add)
                tt = asm.tile(sh, F16, name="tt", tag="tt")
                nc.vector.tensor_mul(tt[sl], den[sl], y0[sl])
                w2 = asm.tile(sh, F16, name="w2", tag="w2")
                nc.vector.tensor_scalar(w2[sl], tt[sl], 2.0, -1.0, AOP.subtract, AOP.mult)
                y1 = asm.tile(sh, F16, name="y1", tag="y1")
                nc.vector.tensor_mul(y1[sl], w2[sl], y0[sl])
                late = nc.gpsimd if j in (1, 2, 3, 4) else nc.vector
                qm = asm.tile(sh, F16, name="qm", tag="qm")
                if s2e is None:
                    nc.vector.tensor_mul(qm[sl], u1[sl], psum_view(ps12[1]))
                else:
                    late.tensor_mul(qm[sl], u1[sl], s2e[sl])
                em = asm.tile(sh, F16, name="em", tag="em")
                late.tensor_add(em[sl], s1e[sl], qm[sl])
                outm = asm.tile(sh, F16, name="outm", tag="outm")
                nc.vector.tensor_mul(outm[sl], em[sl], y1[sl])
                if g is None:
                    nc.sync.dma_start(
                        out=y_d[c0 + 2 : c0 + 2 + n_out], in_=outm[:n_out]
                    )
                else:
                    nc.sync.dma_start(
                        out=y_d[c0 + 2 : c0 + 2 + n_out, GRP * g : GRP * (g + 1)],
                        in_=outm[:n_out, g],
                    )

            for j in range(NCHUNK):
                load_x(j)
            nc.sync.dma_start(out=b16, in_=b16_d[:])
            nc.sync.dma_start(out=b8, in_=b8_d[:])
            for j in range(NCHUNK):
                load_xc(j)
            load_fields(0)
            load_fields(1)
            for j in range(NCHUNK):
                ps0, ps12 = conv_stage(j)
                if j + 2 < NCHUNK:
                    load_fields(j + 2)
                s0e, s1e, s2e = evac_stage(j, ps0, ps12)
                asm_stage(j, s0e, s1e, s2e, ps0, ps12, halves=False)

        if bench_iters == 1:
            body()
        else:
            hints = (
                mybir.EngineType.PE,
                mybir.EngineType.DVE,
                mybir.EngineType.Activation,
                mybir.EngineType.SP,
            )
            with tc.For_i(0, bench_iters, 1, hint_engines=hints):
                body()

    nc.finalize()
    return nc


def _prep_inputs(X: np.ndarray):
    """Full X [4,3,512,512] fp32 -> per-core transposed/padded fp16 arrays."""
    Xr = np.ascontiguousarray(np.asarray(X, dtype=np.float32).reshape(NIMG, H, W))
    b16 = _build_bands16()
    b8 = _build_bands8()
    in_maps = []
    for i in range(N_CORES):
        lo = ROWS * i - 2
        s0, s1 = max(0, lo), min(H, lo + R)
        P = np.zeros((NIMG, R, WPAD), dtype=np.float32)
        P[:, s0 - lo : s1 - lo, 2 : 2 + W] = Xr[:, s0:s1, :]
        xt = np.ascontiguousarray(P.transpose(2, 0, 1)).astype(NP_F16)
        in_maps.append({"x": xt, "b16": b16, "b8": b8})
    return in_maps


_NC_CACHE = {}


def kernel(X: np.ndarray) -> np.ndarray:
    key = (DEGREE, 1)
    if key not in _NC_CACHE:
        _NC_CACHE[key] = build_nc(DEGREE, 1)
    nc = _NC_CACHE[key]
    in_maps = _prep_inputs(X)
    res = run_bass_kernel_spmd(nc, in_maps, list(range(N_CORES)))
    out = np.empty((NIMG, H, W), dtype=np.float32)
    for i in range(N_CORES):
        yi = np.asarray(res.results[i]["y"], dtype=np.float32)  # [WPAD, NIMG, ROWS]
        out[:, ROWS * i : ROWS * (i + 1), :] = 0.5 + yi[2 : 2 + W].transpose(1, 2, 0)
    return out.reshape(4, 3, H, W)
